# revision 1
# baseline (speedup 1.0000x reference)
"""GQA attention forward (B=1, T=2048, DIM=2048, H=16, KV=4, HD=128) on 8 trn2 cores.

Sharding: tensor-parallel over heads. Core c owns q-heads {2c, 2c+1} and kv-head
c//2 (kv work duplicated across the pair of cores sharing it). Each core:
  qT/kT/vT = projections in [hd, t] layout (f32r matmuls, N=512), RoPE on-chip
  for q/k (partition-swap via SBUF-SBUF DMA + sign-folded sin table), v
  PE-transposed to natural [t, hd] layout;
  scores S^T[k, q] = kT-block.T-contract @ qT (over hd), exp on ACT with the
  1/sqrt(hd) folded into the activation scale, causal mask via affine_select
  (fill 0 post-exp);
  A^T[hd, q] accumulates V-block.T-contract @ P^T over k-blocks in PSUM (N=512);
  denominators via ones-matrix MM -> [128, q] (broadcast across partitions);
  A^T normalized by DVE reciprocal+mul; partial out = A^T.T @ woT_c (f32r).
Host: pre-transposes x/weights, sums the 8 partial [T, DIM] outputs.
"""

import sys

if "/opt/trn_rl_repo" not in sys.path:
    sys.path.insert(0, "/opt/trn_rl_repo")

import numpy as np

T = 2048
DIM = 2048
H = 16
KV = 4
HD = 128
NCORES = 8
HPC = H // NCORES            # q heads per core = 2
SCALE = float(HD) ** -0.5
ND = DIM // 128              # dim chunks = 16
NT = T // 128                # t blocks = 16
NQC = T // 512               # q 512-chunks = 4

_CACHE = {}


def _build_nc():
    from contextlib import ExitStack

    from concourse import bacc
    import concourse.mybir as mybir
    import concourse.tile as tile
    from concourse.masks import make_identity

    f32 = mybir.dt.float32
    f32r = mybir.dt.float32r
    Exp = mybir.ActivationFunctionType.Exp

    def r(ap):
        return ap.bitcast(f32r)

    nc = bacc.Bacc("TRN2", target_bir_lowering=False, debug=False,
                   enable_asserts=False)

    xT = nc.dram_tensor("xT", [DIM, T], f32r, kind="ExternalInput").ap()
    wqT = nc.dram_tensor("wqT", [DIM, HPC * HD], f32r, kind="ExternalInput").ap()
    wkT = nc.dram_tensor("wkT", [DIM, HD], f32r, kind="ExternalInput").ap()
    wvT = nc.dram_tensor("wvT", [DIM, HD], f32r, kind="ExternalInput").ap()
    woT = nc.dram_tensor("woT", [HPC * HD, DIM], f32r, kind="ExternalInput").ap()
    cosT = nc.dram_tensor("cosT", [HD, T], f32, kind="ExternalInput").ap()
    sinT = nc.dram_tensor("sinT", [HD, T], f32, kind="ExternalInput").ap()
    out = nc.dram_tensor("out", [T, DIM], f32, kind="ExternalOutput").ap()

    with tile.TileContext(nc) as tc, ExitStack() as ctx:
        const = ctx.enter_context(tc.tile_pool(name="const", bufs=1))
        wpool = ctx.enter_context(tc.tile_pool(name="wts", bufs=1))
        qkv = ctx.enter_context(tc.tile_pool(name="qkv", bufs=1))

        ident = const.tile([128, 128], f32)
        make_identity(nc, ident)
        ones_f = const.tile([128, 128], f32)
        nc.vector.memset(ones_f, 1.0)
        ones_s = const.tile([128, 128], f32r)
        nc.scalar.copy(ones_s, ones_f)

        qT_s = qkv.tile([128, HPC * T], f32r)
        kT_s = qkv.tile([128, T], f32r)
        vT_s = qkv.tile([128, T], f32)
        v_s = qkv.tile([128, NT * HD], f32r)   # natural [t%128, hd] per t-block

        # ---- Phase 1: projections + RoPE + v-transpose, four t-quarters ----
        with tc.tile_pool(name="xp", bufs=18) as xpool, \
             tc.tile_pool(name="rope", bufs=4) as rp, \
             tc.tile_pool(name="vtp", bufs=3, space="PSUM") as vtp, \
             tc.tile_pool(name="p1ps", bufs=3, space="PSUM") as ps1:

            def load_x_quarter(tq):
                xts = []
                for d in range(ND):
                    xt = xpool.tile([128, 512], f32r, tag="xt",
                                    name=f"xt{tq}_{d}")
                    nc.sync.dma_start(
                        xt, xT[d * 128:(d + 1) * 128,
                               tq * 512:(tq + 1) * 512])
                    xts.append(xt)
                return xts

            wk_s = wpool.tile([128, ND, HD], f32r)
            nc.sync.dma_start(wk_s, wkT.rearrange("(d p) n -> p d n", p=128))
            xq = [load_x_quarter(0)]
            wq_s = wpool.tile([128, ND, HPC * HD], f32r)
            nc.sync.dma_start(wq_s, wqT.rearrange("(d p) n -> p d n", p=128))
            wv_s = wpool.tile([128, ND, HD], f32r)
            nc.sync.dma_start(wv_s, wvT.rearrange("(d p) n -> p d n", p=128))
            cos_s = const.tile([128, T], f32)
            nc.sync.dma_start(cos_s, cosT)
            sin_s = const.tile([128, T], f32)
            nc.sync.dma_start(sin_s, sinT)
            xq.append(load_x_quarter(1))

            def rope(u, c0, t0, cols=512):
                us = u[:, c0:c0 + cols]
                rot = rp.tile([128, cols], f32r, tag="rot")
                nc.sync.dma_start(rot[0:64, :], us[64:128, :])
                nc.sync.dma_start(rot[64:128, :], us[0:64, :])
                tmp = rp.tile([128, cols], f32, tag="rtmp")
                nc.vector.tensor_mul(tmp, us, cos_s[:, t0:t0 + cols])
                nc.vector.tensor_mul(rot, rot, sin_s[:, t0:t0 + cols])
                nc.vector.tensor_add(us, tmp, rot)

            def proj(acc_tag, w_ap, xts, dst, c0):
                acc = ps1.tile([128, 512], f32, tag="pps", name=acc_tag)
                for d in range(ND):
                    nc.tensor.matmul(acc, w_ap(d), r(xts[d]),
                                     start=(d == 0), stop=(d == ND - 1))
                nc.scalar.copy(dst[:, c0:c0 + 512], acc)

            for tq in range(4):
                if tq + 2 <= 3:
                    xq.append(load_x_quarter(tq + 2))
                xts = xq[tq]
                t0 = tq * 512
                proj("k", lambda d: r(wk_s[:, d, :]), xts, kT_s, t0)
                rope(kT_s, t0, t0)
                for h in range(HPC):
                    proj(f"q{h}",
                         lambda d, h=h: r(wq_s[:, d, h * HD:(h + 1) * HD]),
                         xts, qT_s, h * T + t0)
                    rope(qT_s, h * T + t0, t0)
                proj("v", lambda d: r(wv_s[:, d, :]), xts, vT_s, t0)
                for tb in range(tq * 4, tq * 4 + 4):
                    vt = vtp.tile([128, 128], f32, tag="vt")
                    nc.tensor.transpose(
                        vt, vT_s[:, tb * 128:(tb + 1) * 128], ident)
                    nc.scalar.copy(
                        v_s[:, tb * HD:(tb + 1) * HD], vt)

        # ---- Phase 2+3: attention (1-unit scores lookahead), wo interleaved ----
        apool = ctx.enter_context(tc.tile_pool(name="Apool", bufs=1))
        aT_s = [apool.tile([128, T], f32r, name=f"aT{h}") for h in range(HPC)]
        units = [(h, qc) for h in range(HPC) for qc in range(NQC)]

        with tc.tile_pool(name="sps", bufs=2, space="PSUM") as sps, \
             tc.tile_pool(name="otp", bufs=2, space="PSUM") as otp, \
             tc.tile_pool(name="dnp", bufs=2, space="PSUM") as dnp, \
             tc.tile_pool(name="pp", bufs=30) as ppool, \
             tc.tile_pool(name="rcp", bufs=2) as rpool, \
             tc.tile_pool(name="wops", bufs=2, space="PSUM") as wops, \
             tc.tile_pool(name="ost", bufs=8) as ostage:
            wo_s = wpool.tile([128, HPC, DIM], f32r)
            nc.sync.dma_start(wo_s, woT.rearrange("(h p) n -> p h n", p=128))

            def scores_burst(u):
                h, qc = units[u]
                qTh = qT_s[:, h * T:(h + 1) * T]
                nkb = 4 * qc + 4
                ptiles = []
                for kb in range(nkb):
                    s_ps = sps.tile([128, 512], f32, tag="s",
                                    name=f"s{u}_{kb}")
                    nc.tensor.matmul(
                        s_ps, r(kT_s[:, kb * 128:(kb + 1) * 128]),
                        r(qTh[:, qc * 512:(qc + 1) * 512]),
                        start=True, stop=True)
                    p_sb = ppool.tile([128, 512], f32r, tag="p",
                                      name=f"p{u}_{kb}")
                    nc.scalar.activation(p_sb, s_ps, Exp, scale=SCALE)
                    if kb >= 4 * qc:
                        nc.gpsimd.affine_select(
                            out=p_sb, in_=p_sb,
                            compare_op=mybir.AluOpType.is_ge,
                            fill=0.0, base=qc * 512 - kb * 128,
                            channel_multiplier=-1, pattern=[[1, 512]])
                    ptiles.append(p_sb)
                return ptiles

            def av_burst(u, ptiles):
                h, qc = units[u]
                nkb = 4 * qc + 4
                oT = otp.tile([128, 512], f32, tag="oT", name=f"oT{u}")
                dn = dnp.tile([128, 512], f32, tag="dn", name=f"dn{u}")
                for kb in range(nkb):
                    nc.tensor.matmul(
                        oT, r(v_s[:, kb * HD:(kb + 1) * HD]), r(ptiles[kb]),
                        start=(kb == 0), stop=(kb == nkb - 1))
                for kb in range(nkb):
                    nc.tensor.matmul(
                        dn, r(ones_s), r(ptiles[kb]),
                        start=(kb == 0), stop=(kb == nkb - 1))
                rec = rpool.tile([128, 512], f32, tag="rec")
                nc.vector.reciprocal(rec, dn)
                nc.vector.tensor_mul(
                    aT_s[h][:, qc * 512:(qc + 1) * 512], oT, rec)

            def wo_block(qc):
                for tb in range(qc * 4, qc * 4 + 4):
                    for n4 in range(4):
                        op = wops.tile([128, 512], f32, tag="op")
                        for h in range(HPC):
                            nc.tensor.matmul(
                                op, r(aT_s[h][:, tb * 128:(tb + 1) * 128]),
                                r(wo_s[:, h, n4 * 512:(n4 + 1) * 512]),
                                start=(h == 0), stop=(h == HPC - 1))
                        ob = ostage.tile([128, 512], f32, tag="ob")
                        nc.vector.tensor_copy(ob, op)
                        nc.sync.dma_start(
                            out[tb * 128:(tb + 1) * 128,
                                n4 * 512:(n4 + 1) * 512], ob)

            pending = scores_burst(0)
            for u in range(len(units)):
                nxt = scores_burst(u + 1) if u + 1 < len(units) else None
                av_burst(u, pending)
                pending = nxt
            for qc in range(NQC):
                wo_block(qc)

    nc.compile()
    return nc


def _shard_inputs(x, wq, wk, wv, wo, cos, sin):
    xTh = np.ascontiguousarray(x.reshape(T, DIM).T)
    cosTh = np.ascontiguousarray(cos.T)
    # rotate_half sign fold: out = u*cos + u_rot*sin_signed
    sinTh = np.ascontiguousarray(sin.T).copy()
    sinTh[: HD // 2, :] *= -1.0
    in_maps = []
    for c in range(NCORES):
        g = c // 2
        in_maps.append({
            "xT": xTh,
            "wqT": np.ascontiguousarray(
                wq[c * HPC * HD:(c + 1) * HPC * HD, :].T),
            "wkT": np.ascontiguousarray(wk[g * HD:(g + 1) * HD, :].T),
            "wvT": np.ascontiguousarray(wv[g * HD:(g + 1) * HD, :].T),
            "woT": np.ascontiguousarray(
                wo[:, c * HPC * HD:(c + 1) * HPC * HD].T),
            "cosT": cosTh,
            "sinT": sinTh,
        })
    return in_maps


def _get_exec():
    """Build (once) a cached jitted SPMD executable over the 8 cores.

    Mirrors bass2jax.run_bass_via_pjrt's multi-core branch, but caches the
    jitted callable so repeat kernel() calls don't re-trace/re-lower.
    """
    if "exec" in _CACHE:
        return _CACHE["exec"]

    import jax
    from jax.sharding import Mesh, PartitionSpec
    from jax.experimental.shard_map import shard_map
    from concourse import bass2jax
    import concourse.mybir as mybir

    if "nc" not in _CACHE:
        _CACHE["nc"] = _build_nc()
    nc = _CACHE["nc"]

    bass2jax.install_neuronx_cc_hook()

    part_name = (nc.partition_id_tensor.name
                 if nc.partition_id_tensor else None)
    in_names, out_names, out_avals = [], [], []
    for alloc in nc.m.functions[0].allocations:
        if not isinstance(alloc, mybir.MemoryLocationSet):
            continue
        name = alloc.memorylocations[0].name
        if alloc.kind == "ExternalInput":
            if name != part_name:
                in_names.append(name)
        elif alloc.kind == "ExternalOutput":
            out_names.append(name)
            out_avals.append(jax.core.ShapedArray(
                tuple(alloc.tensor_shape), mybir.dt.np(alloc.dtype)))

    bind_names = in_names + out_names
    if part_name is not None:
        bind_names = bind_names + [part_name]

    def _body(*args):
        operands = list(args)
        if part_name is not None:
            operands.append(bass2jax.partition_id_tensor())
        outs = bass2jax._bass_exec_p.bind(
            *operands,
            out_avals=tuple(out_avals),
            in_names=tuple(bind_names),
            out_names=tuple(out_names),
            lowering_input_output_aliases=(),
            sim_require_finite=True,
            sim_require_nnan=True,
            nc=nc,
        )
        return tuple(outs)

    devices = jax.devices()[:NCORES]
    mesh = Mesh(np.asarray(devices), ("core",))
    n_in = len(in_names)
    n_out = len(out_names)
    sharded = jax.jit(
        shard_map(
            _body, mesh=mesh,
            in_specs=(PartitionSpec("core"),) * (n_in + n_out),
            out_specs=(PartitionSpec("core"),) * n_out,
            check_rep=False,
        ),
        donate_argnums=tuple(range(n_in, n_in + n_out)),
        keep_unused=True,
    )
    _CACHE["body"] = _body
    _CACHE["exec"] = (sharded, in_names, out_names, out_avals, mesh)
    return _CACHE["exec"]


def _concat_inputs(in_maps, in_names):
    return [
        np.concatenate([in_maps[c][name] for c in range(NCORES)], axis=0)
        for name in in_names
    ]


def _zero_outs(out_avals):
    return [
        np.zeros((NCORES * a.shape[0], *a.shape[1:]), a.dtype)
        for a in out_avals
    ]


def kernel(**inputs):
    sharded, in_names, out_names, out_avals, _ = _get_exec()

    in_maps = _shard_inputs(
        np.asarray(inputs["x"], dtype=np.float32),
        np.asarray(inputs["wq"], dtype=np.float32),
        np.asarray(inputs["wk"], dtype=np.float32),
        np.asarray(inputs["wv"], dtype=np.float32),
        np.asarray(inputs["wo"], dtype=np.float32),
        np.asarray(inputs["cos"], dtype=np.float32),
        np.asarray(inputs["sin"], dtype=np.float32),
    )
    concat_in = _concat_inputs(in_maps, in_names)
    out_arrs = sharded(*concat_in, *_zero_outs(out_avals))

    full = np.asarray(out_arrs[out_names.index("out")])
    acc = full.reshape(NCORES, T, DIM).astype(np.float32).sum(axis=0)
    return acc.reshape(1, T, DIM)



# revision 16
# speedup vs baseline: 385.9643x; 385.9643x over previous
"""GQA attention forward (B=1, T=2048, DIM=2048, H=16, KV=4, HD=128) on 8 trn2 cores.

Sharding: tensor-parallel over heads. Core c owns q-heads {2c, 2c+1} and kv-head
c//2 (kv work duplicated across the pair of cores sharing it).

v2: all-bf16 matmul operands (f32r moving operands stream at 2 cyc/row on HW;
bf16 streams at 1 cyc/row, halving tensor-engine time), per-quarter interleaved
emission so projections, attention, and wo output pipeline across engines,
reciprocal_approx_fast for the softmax denominators, bf16 DMA in/out (halves
HBM traffic). Accumulation stays f32 in PSUM; rel err ~1e-3 vs f32 reference.

Per core:
  qT/kT/vT projections in [hd, t] layout (bf16 MMs, N=512 moving), RoPE on-chip
  (partition-swap via SBUF-SBUF DMA + sign-folded sin table), v PE-transposed
  to natural [t, hd] layout;
  scores S^T[k, q] = kT-block stationary @ qT moving (contract hd), exp on ACT
  with 1/sqrt(hd) folded into activation scale, causal mask via gpsimd
  affine_select (fill 0 post-exp);
  A^T[hd, q] and denominators accumulate over k-blocks in PSUM;
  aT normalized by DVE reciprocal_approx_fast + mul; partial out = aT.T @ woT.
Host: pre-transposes + bf16-casts inputs, sums the 8 partial [T, DIM] outputs.
"""

import sys

if "/opt/trn_rl_repo" not in sys.path:
    sys.path.insert(0, "/opt/trn_rl_repo")

import numpy as np

T = 2048
DIM = 2048
H = 16
KV = 4
HD = 128
NCORES = 8
HPC = H // NCORES            # q heads per core = 2
SCALE = float(HD) ** -0.5
ND = DIM // 128              # dim chunks = 16
NT = T // 128                # t blocks = 16
NQC = T // 512               # q 512-chunks = 4

_CACHE = {}


def _build_nc():
    from contextlib import ExitStack

    from concourse import bacc
    import concourse.mybir as mybir
    import concourse.tile as tile
    from concourse.masks import make_identity

    f32 = mybir.dt.float32
    bf16 = mybir.dt.bfloat16
    Exp = mybir.ActivationFunctionType.Exp

    nc = bacc.Bacc("TRN2", target_bir_lowering=False, debug=False,
                   enable_asserts=False)

    xT = nc.dram_tensor("xT", [DIM, T], bf16, kind="ExternalInput").ap()
    wqT = nc.dram_tensor("wqT", [DIM, HPC * HD], bf16, kind="ExternalInput").ap()
    wkT = nc.dram_tensor("wkT", [DIM, HD], bf16, kind="ExternalInput").ap()
    wvT = nc.dram_tensor("wvT", [DIM, HD], bf16, kind="ExternalInput").ap()
    woT = nc.dram_tensor("woT", [HPC * HD, DIM], bf16, kind="ExternalInput").ap()
    cosT = nc.dram_tensor("cosT", [HD, T], bf16, kind="ExternalInput").ap()
    sinT = nc.dram_tensor("sinT", [HD, T], bf16, kind="ExternalInput").ap()
    out = nc.dram_tensor("out", [T, DIM], bf16, kind="ExternalOutput").ap()

    with tile.TileContext(nc) as tc, ExitStack() as ctx:
        const = ctx.enter_context(tc.tile_pool(name="const", bufs=1))
        wpool = ctx.enter_context(tc.tile_pool(name="wts", bufs=1))
        qkv = ctx.enter_context(tc.tile_pool(name="qkv", bufs=1))

        ones_s = const.tile([128, 128], bf16)
        nc.vector.memset(ones_s, 1.0)

        qT_s = qkv.tile([128, HPC * T], bf16)
        kT_s = qkv.tile([128, T], bf16)
        vT_s = qkv.tile([128, T], bf16)
        v_s = qkv.tile([128, NT * HD], bf16)   # natural [t%128, hd] per t-block
        aT_s = [qkv.tile([128, T], bf16, name=f"aT{h}") for h in range(HPC)]

        xpool = ctx.enter_context(tc.tile_pool(name="xp", bufs=32))
        rp = ctx.enter_context(tc.tile_pool(name="rope", bufs=4))
        ppool = ctx.enter_context(tc.tile_pool(name="pp", bufs=34))
        rpool = ctx.enter_context(tc.tile_pool(name="rcp", bufs=2))
        ostage = ctx.enter_context(tc.tile_pool(name="ost", bufs=8))

        ps1 = ctx.enter_context(tc.tile_pool(name="p1ps", bufs=2, space="PSUM"))
        sps = ctx.enter_context(tc.tile_pool(name="sps", bufs=2, space="PSUM"))
        otp = ctx.enter_context(tc.tile_pool(name="otp", bufs=1, space="PSUM"))
        dnp = ctx.enter_context(tc.tile_pool(name="dnp", bufs=1, space="PSUM"))
        wops = ctx.enter_context(tc.tile_pool(name="wops", bufs=2, space="PSUM"))

        def load_x_quarter(tq):
            # two d-chunks per [128, 1024] tile: halves DMA descriptor count.
            # Descriptor gen on the (otherwise idle early) gpsimd queue.
            xts = []
            for d2 in range(ND // 2):
                xt = xpool.tile([128, 2, 512], bf16, tag="xt",
                                name=f"xt{tq}_{d2}")
                nc.gpsimd.dma_start(
                    xt, xT[d2 * 256:(d2 + 1) * 256,
                           tq * 512:(tq + 1) * 512].rearrange(
                               "(two p) n -> p two n", p=128))
                xts.append(xt)
            return [xts[d // 2][:, d % 2, :] for d in range(ND)]

        # weights arrive host-prearranged in [p, d, n] SBUF layout (straight
        # contiguous DMA; the on-the-fly rearrange gather was ~10us)
        wk_s = wpool.tile([128, ND, HD], bf16)
        nc.gpsimd.dma_start(wk_s, wkT.rearrange("(p d) n -> p d n", p=128))
        xq = [load_x_quarter(0)]
        wq_s = wpool.tile([128, ND, HPC * HD], bf16)
        nc.gpsimd.dma_start(wq_s, wqT.rearrange("(p d) n -> p d n", p=128))
        wv_s = wpool.tile([128, ND, HD], bf16)
        nc.gpsimd.dma_start(wv_s, wvT.rearrange("(p d) n -> p d n", p=128))
        cos_s = const.tile([128, T], bf16)
        nc.gpsimd.dma_start(cos_s, cosT)
        sin_s = const.tile([128, T], bf16)
        nc.gpsimd.dma_start(sin_s, sinT)
        wo_s = wpool.tile([128, HPC, DIM], bf16)
        nc.gpsimd.dma_start(wo_s, woT.rearrange("(p h) n -> p h n", p=128))

        # PE warm-up: dense dummy matmuls during the DMA head so the HAM
        # clock gate releases (1.2 -> 2.4 GHz) before the real work lands
        warm = ps1.tile([128, 512], f32, tag="pps", name="warm")
        for i in range(40):
            nc.tensor.matmul(warm[:, 0:128], ones_s, ones_s,
                             start=(i == 0), stop=(i == 39))
        wsink = rp.tile([128, 128], bf16, tag="rot", name="wsink")
        nc.scalar.copy(wsink, warm[:, 0:128])

        def rope(u, c0, t0, cols=512):
            us = u[:, c0:c0 + cols]
            rot = rp.tile([128, cols], bf16, tag="rot")
            nc.sync.dma_start(rot[0:64, :], us[64:128, :])
            nc.sync.dma_start(rot[64:128, :], us[0:64, :])
            tmp = rp.tile([128, cols], bf16, tag="rtmp")
            nc.vector.tensor_mul(tmp, us, cos_s[:, t0:t0 + cols])
            nc.vector.tensor_mul(rot, rot, sin_s[:, t0:t0 + cols])
            nc.vector.tensor_add(us, tmp, rot)

        def proj(acc_tag, w_ap, xts, dst, c0):
            acc = ps1.tile([128, 512], f32, tag="pps", name=acc_tag)
            for d in range(ND):
                nc.tensor.matmul(acc, w_ap(d), xts[d],
                                 start=(d == 0), stop=(d == ND - 1))
            nc.scalar.copy(dst[:, c0:c0 + 512], acc)

        def scores_burst(h, qc):
            # kb high->low: diagonal (masked) tiles first so their gpsimd
            # affine_select latency hides behind the remaining exps
            qTh = qT_s[:, h * T + qc * 512:h * T + (qc + 1) * 512]
            nkb = 4 * qc + 4
            ptiles = {}
            for kb in reversed(range(nkb)):
                s_ps = sps.tile([128, 512], f32, tag="s",
                                name=f"s{h}_{qc}_{kb}")
                nc.tensor.matmul(
                    s_ps, kT_s[:, kb * 128:(kb + 1) * 128], qTh,
                    start=True, stop=True)
                p_sb = ppool.tile([128, 512], bf16, tag="p",
                                  name=f"p{h}_{qc}_{kb}")
                nc.scalar.activation(p_sb, s_ps, Exp, scale=SCALE)
                if kb >= 4 * qc:
                    nc.gpsimd.affine_select(
                        out=p_sb, in_=p_sb,
                        compare_op=mybir.AluOpType.is_ge,
                        fill=0.0, base=qc * 512 - kb * 128,
                        channel_multiplier=-1, pattern=[[1, 512]])
                ptiles[kb] = p_sb
            return ptiles

        def av_burst(h, qc, ptiles):
            nkb = 4 * qc + 4
            order = list(reversed(range(nkb)))
            oT = otp.tile([128, 512], f32, tag="oT", name=f"oT{h}_{qc}")
            dn = dnp.tile([128, 512], f32, tag="dn", name=f"dn{h}_{qc}")
            for i, kb in enumerate(order):
                nc.tensor.matmul(
                    oT, v_s[:, kb * HD:(kb + 1) * HD], ptiles[kb],
                    start=(i == 0), stop=(i == nkb - 1))
            for i, kb in enumerate(order):
                nc.tensor.matmul(
                    dn, ones_s, ptiles[kb],
                    start=(i == 0), stop=(i == nkb - 1))
            rec = rpool.tile([128, 512], f32, tag="rec")
            nc.vector.reciprocal_approx_fast(rec, dn)
            nc.vector.tensor_mul(
                aT_s[h][:, qc * 512:(qc + 1) * 512], oT, rec)

        def wo_block(qc):
            for tb in range(qc * 4, qc * 4 + 4):
                for n4 in range(4):
                    op = wops.tile([128, 512], f32, tag="op")
                    for h in range(HPC):
                        nc.tensor.matmul(
                            op, aT_s[h][:, tb * 128:(tb + 1) * 128],
                            wo_s[:, h, n4 * 512:(n4 + 1) * 512],
                            start=(h == 0), stop=(h == HPC - 1))
                    ob = ostage.tile([128, 512], bf16, tag="ob")
                    if (tb * 4 + n4) % 4 == 3:
                        nc.scalar.copy(ob, op)
                    else:
                        nc.vector.tensor_copy(ob, op)
                    nc.sync.dma_start(
                        out[tb * 128:(tb + 1) * 128,
                            n4 * 512:(n4 + 1) * 512], ob)

        # All x quarters preloaded up front (32 bufs, no slot recycling):
        # transfers drain the DMA engines by ~30us, before the v-transpose
        # DMAs (whose xbar-mode switch serializes the shared engines).
        for tq in range(1, 4):
            xq.append(load_x_quarter(tq))

        # Software pipeline with one-quarter lag between phases: proj
        # copies of quarter s queue ahead of quarter s-1's exps on ACT,
        # wo of s-2 rides along; x is fully preloaded so the v-transpose
        # DMAs (xbar-mode serialization) never stall the loads.
        for s in range(6):
            if s < 4:
                xts = xq[s]
                t0 = s * 512
                proj("k", lambda d: wk_s[:, d, :], xts, kT_s, t0)
                rope(kT_s, t0, t0)
                for h in range(HPC):
                    proj(f"q{h}",
                         lambda d, h=h: wq_s[:, d, h * HD:(h + 1) * HD],
                         xts, qT_s, h * T + t0)
                    rope(qT_s, h * T + t0, t0)
                proj("v", lambda d: wv_s[:, d, :], xts, vT_s, t0)
                for tb in range(s * 4, s * 4 + 4):
                    nc.sync.dma_start_transpose(
                        v_s[:, tb * HD:(tb + 1) * HD],
                        vT_s[:, tb * 128:(tb + 1) * 128])
            if 1 <= s <= 4:
                qc = s - 1
                pending = scores_burst(0, qc)
                for h in range(HPC):
                    nxt = scores_burst(1, qc) if h == 0 else None
                    av_burst(h, qc, pending)
                    pending = nxt
            if s >= 2:
                wo_block(s - 2)

    nc.compile()
    return nc


def _shard_inputs(x, wq, wk, wv, wo, cos, sin):
    import ml_dtypes
    bf = ml_dtypes.bfloat16

    def pdn(wT, n):
        # [DIM, n] row-major (d p) -> (p d): straight per-partition DMA bursts
        return np.ascontiguousarray(
            wT.reshape(ND, 128, n).transpose(1, 0, 2).reshape(DIM, n)
        ).astype(bf)

    xTh = np.ascontiguousarray(x.reshape(T, DIM).T).astype(bf)
    cosTh = np.ascontiguousarray(cos.T).astype(bf)
    # rotate_half sign fold: out = u*cos + u_rot*sin_signed
    sinTh = np.ascontiguousarray(sin.T).copy()
    sinTh[: HD // 2, :] *= -1.0
    sinTh = sinTh.astype(bf)
    in_maps = []
    for c in range(NCORES):
        g = c // 2
        woTc = wo[:, c * HPC * HD:(c + 1) * HPC * HD].T  # [(h p), DIM]
        in_maps.append({
            "xT": xTh,
            "wqT": pdn(wq[c * HPC * HD:(c + 1) * HPC * HD, :].T, HPC * HD),
            "wkT": pdn(wk[g * HD:(g + 1) * HD, :].T, HD),
            "wvT": pdn(wv[g * HD:(g + 1) * HD, :].T, HD),
            "woT": np.ascontiguousarray(
                woTc.reshape(HPC, 128, DIM).transpose(1, 0, 2).reshape(
                    HPC * HD, DIM)).astype(bf),
            "cosT": cosTh,
            "sinT": sinTh,
        })
    return in_maps


def _get_exec():
    """Build (once) a cached jitted SPMD executable over the 8 cores.

    Mirrors bass2jax.run_bass_via_pjrt's multi-core branch, but caches the
    jitted callable so repeat kernel() calls don't re-trace/re-lower.
    """
    if "exec" in _CACHE:
        return _CACHE["exec"]

    import jax
    from jax.sharding import Mesh, PartitionSpec
    from jax.experimental.shard_map import shard_map
    from concourse import bass2jax
    import concourse.mybir as mybir

    if "nc" not in _CACHE:
        _CACHE["nc"] = _build_nc()
    nc = _CACHE["nc"]

    bass2jax.install_neuronx_cc_hook()

    part_name = (nc.partition_id_tensor.name
                 if nc.partition_id_tensor else None)
    in_names, out_names, out_avals = [], [], []
    for alloc in nc.m.functions[0].allocations:
        if not isinstance(alloc, mybir.MemoryLocationSet):
            continue
        name = alloc.memorylocations[0].name
        if alloc.kind == "ExternalInput":
            if name != part_name:
                in_names.append(name)
        elif alloc.kind == "ExternalOutput":
            out_names.append(name)
            out_avals.append(jax.core.ShapedArray(
                tuple(alloc.tensor_shape), mybir.dt.np(alloc.dtype)))

    bind_names = in_names + out_names
    if part_name is not None:
        bind_names = bind_names + [part_name]

    def _body(*args):
        operands = list(args)
        if part_name is not None:
            operands.append(bass2jax.partition_id_tensor())
        outs = bass2jax._bass_exec_p.bind(
            *operands,
            out_avals=tuple(out_avals),
            in_names=tuple(bind_names),
            out_names=tuple(out_names),
            lowering_input_output_aliases=(),
            sim_require_finite=True,
            sim_require_nnan=True,
            nc=nc,
        )
        return tuple(outs)

    devices = jax.devices()[:NCORES]
    mesh = Mesh(np.asarray(devices), ("core",))
    n_in = len(in_names)
    n_out = len(out_names)
    sharded = jax.jit(
        shard_map(
            _body, mesh=mesh,
            in_specs=(PartitionSpec("core"),) * (n_in + n_out),
            out_specs=(PartitionSpec("core"),) * n_out,
            check_rep=False,
        ),
        donate_argnums=tuple(range(n_in, n_in + n_out)),
        keep_unused=True,
    )
    _CACHE["body"] = _body
    _CACHE["exec"] = (sharded, in_names, out_names, out_avals, mesh)
    return _CACHE["exec"]


def _concat_inputs(in_maps, in_names):
    return [
        np.concatenate([in_maps[c][name] for c in range(NCORES)], axis=0)
        for name in in_names
    ]


def _zero_outs(out_avals):
    return [
        np.zeros((NCORES * a.shape[0], *a.shape[1:]), a.dtype)
        for a in out_avals
    ]


def kernel(**inputs):
    sharded, in_names, out_names, out_avals, _ = _get_exec()

    in_maps = _shard_inputs(
        np.asarray(inputs["x"], dtype=np.float32),
        np.asarray(inputs["wq"], dtype=np.float32),
        np.asarray(inputs["wk"], dtype=np.float32),
        np.asarray(inputs["wv"], dtype=np.float32),
        np.asarray(inputs["wo"], dtype=np.float32),
        np.asarray(inputs["cos"], dtype=np.float32),
        np.asarray(inputs["sin"], dtype=np.float32),
    )
    concat_in = _concat_inputs(in_maps, in_names)
    out_arrs = sharded(*concat_in, *_zero_outs(out_avals))

    full = np.asarray(out_arrs[out_names.index("out")])
    acc = full.reshape(NCORES, T, DIM).astype(np.float32).sum(axis=0)
    return acc.reshape(1, T, DIM)


# revision 21
# speedup vs baseline: 389.7095x; 1.0097x over previous
"""GQA attention forward (B=1, T=2048, DIM=2048, H=16, KV=4, HD=128) on 8 trn2 cores.

Sharding: tensor-parallel over heads. Core c owns q-heads {2c, 2c+1} and kv-head
c//2 (kv work duplicated across the pair of cores sharing it).

v2: all-bf16 matmul operands (f32r moving operands stream at 2 cyc/row on HW;
bf16 streams at 1 cyc/row, halving tensor-engine time), per-quarter interleaved
emission so projections, attention, and wo output pipeline across engines,
reciprocal_approx_fast for the softmax denominators, bf16 DMA in/out (halves
HBM traffic). Accumulation stays f32 in PSUM; rel err ~1e-3 vs f32 reference.

Per core:
  qT/kT/vT projections in [hd, t] layout (bf16 MMs, N=512 moving), RoPE on-chip
  (partition-swap via SBUF-SBUF DMA + sign-folded sin table), v PE-transposed
  to natural [t, hd] layout;
  scores S^T[k, q] = kT-block stationary @ qT moving (contract hd), exp on ACT
  with 1/sqrt(hd) folded into activation scale, causal mask via gpsimd
  affine_select (fill 0 post-exp);
  A^T[hd, q] and denominators accumulate over k-blocks in PSUM;
  aT normalized by DVE reciprocal_approx_fast + mul; partial out = aT.T @ woT.
Host: pre-transposes + bf16-casts inputs, sums the 8 partial [T, DIM] outputs.
"""

import sys

if "/opt/trn_rl_repo" not in sys.path:
    sys.path.insert(0, "/opt/trn_rl_repo")

import numpy as np

T = 2048
DIM = 2048
H = 16
KV = 4
HD = 128
NCORES = 8
HPC = H // NCORES            # q heads per core = 2
SCALE = float(HD) ** -0.5
ND = DIM // 128              # dim chunks = 16
NT = T // 128                # t blocks = 16
NQC = T // 512               # q 512-chunks = 4

_CACHE = {}


def _build_nc():
    from contextlib import ExitStack

    from concourse import bacc
    import concourse.mybir as mybir
    import concourse.tile as tile
    from concourse.masks import make_identity

    f32 = mybir.dt.float32
    bf16 = mybir.dt.bfloat16
    Exp = mybir.ActivationFunctionType.Exp

    nc = bacc.Bacc("TRN2", target_bir_lowering=False, debug=False,
                   enable_asserts=False)

    xT = nc.dram_tensor("xT", [DIM, T], bf16, kind="ExternalInput").ap()
    wqT = nc.dram_tensor("wqT", [DIM, HPC * HD], bf16, kind="ExternalInput").ap()
    wkT = nc.dram_tensor("wkT", [DIM, HD], bf16, kind="ExternalInput").ap()
    wvT = nc.dram_tensor("wvT", [DIM, HD], bf16, kind="ExternalInput").ap()
    woT = nc.dram_tensor("woT", [HPC * HD, DIM], bf16, kind="ExternalInput").ap()
    cosT = nc.dram_tensor("cosT", [HD, T], bf16, kind="ExternalInput").ap()
    sinT = nc.dram_tensor("sinT", [HD, T], bf16, kind="ExternalInput").ap()
    out = nc.dram_tensor("out", [T, DIM], bf16, kind="ExternalOutput").ap()

    with tile.TileContext(nc) as tc, ExitStack() as ctx:
        const = ctx.enter_context(tc.tile_pool(name="const", bufs=1))
        wpool = ctx.enter_context(tc.tile_pool(name="wts", bufs=1))
        qkv = ctx.enter_context(tc.tile_pool(name="qkv", bufs=1))

        ones_s = const.tile([128, 128], bf16)
        nc.vector.memset(ones_s, 1.0)
        ident = const.tile([128, 128], bf16)
        make_identity(nc, ident)

        qT_s = qkv.tile([128, HPC * T], bf16)
        kT_s = qkv.tile([128, T], bf16)
        vT_s = qkv.tile([128, T], bf16)
        v_s = qkv.tile([128, NT * HD], bf16)   # natural [t%128, hd] per t-block
        aT_s = [qkv.tile([128, T], bf16, name=f"aT{h}") for h in range(HPC)]

        xpool = ctx.enter_context(tc.tile_pool(name="xp", bufs=32))
        rp = ctx.enter_context(tc.tile_pool(name="rope", bufs=4))
        ppool = ctx.enter_context(tc.tile_pool(name="pp", bufs=34))
        rpool = ctx.enter_context(tc.tile_pool(name="rcp", bufs=2))
        ostage = ctx.enter_context(tc.tile_pool(name="ost", bufs=8))

        ps1 = ctx.enter_context(tc.tile_pool(name="p1ps", bufs=1, space="PSUM"))
        vtp = ctx.enter_context(tc.tile_pool(name="vtp", bufs=1, space="PSUM"))
        sps = ctx.enter_context(tc.tile_pool(name="sps", bufs=2, space="PSUM"))
        otp = ctx.enter_context(tc.tile_pool(name="otp", bufs=1, space="PSUM"))
        dnp = ctx.enter_context(tc.tile_pool(name="dnp", bufs=1, space="PSUM"))
        wops = ctx.enter_context(tc.tile_pool(name="wops", bufs=2, space="PSUM"))

        def load_x_quarter(tq):
            # two d-chunks per [128, 1024] tile: halves DMA descriptor count.
            # Descriptor gen on the (otherwise idle early) gpsimd queue.
            xts = []
            for d2 in range(ND // 2):
                xt = xpool.tile([128, 2, 512], bf16, tag="xt",
                                name=f"xt{tq}_{d2}")
                nc.gpsimd.dma_start(
                    xt, xT[d2 * 256:(d2 + 1) * 256,
                           tq * 512:(tq + 1) * 512].rearrange(
                               "(two p) n -> p two n", p=128))
                xts.append(xt)
            return [xts[d // 2][:, d % 2, :] for d in range(ND)]

        # weights arrive host-prearranged in [p, d, n] SBUF layout (straight
        # contiguous DMA; the on-the-fly rearrange gather was ~10us)
        wk_s = wpool.tile([128, ND, HD], bf16)
        nc.gpsimd.dma_start(wk_s, wkT.rearrange("(p d) n -> p d n", p=128))
        xq = [load_x_quarter(0)]
        wq_s = wpool.tile([128, ND, HPC * HD], bf16)
        nc.gpsimd.dma_start(wq_s, wqT.rearrange("(p d) n -> p d n", p=128))
        wv_s = wpool.tile([128, ND, HD], bf16)
        nc.gpsimd.dma_start(wv_s, wvT.rearrange("(p d) n -> p d n", p=128))
        cos_s = const.tile([128, T], bf16)
        nc.gpsimd.dma_start(cos_s, cosT)
        sin_s = const.tile([128, T], bf16)
        nc.gpsimd.dma_start(sin_s, sinT)
        wo_s = wpool.tile([128, HPC, DIM], bf16)
        nc.gpsimd.dma_start(wo_s, woT.rearrange("(p h) n -> p h n", p=128))

        # PE warm-up: dense dummy matmuls during the DMA head so the HAM
        # clock gate releases (1.2 -> 2.4 GHz) before the real work lands
        warm = ps1.tile([128, 512], f32, tag="pps", name="warm")
        for i in range(40):
            nc.tensor.matmul(warm[:, 0:128], ones_s, ones_s,
                             start=(i == 0), stop=(i == 39))
        wsink = rp.tile([128, 128], bf16, tag="rot", name="wsink")
        nc.scalar.copy(wsink, warm[:, 0:128])

        def rope(u, c0, t0, cols=512):
            us = u[:, c0:c0 + cols]
            rot = rp.tile([128, cols], bf16, tag="rot")
            nc.sync.dma_start(rot[0:64, :], us[64:128, :])
            nc.sync.dma_start(rot[64:128, :], us[0:64, :])
            tmp = rp.tile([128, cols], bf16, tag="rtmp")
            nc.vector.tensor_mul(tmp, us, cos_s[:, t0:t0 + cols])
            nc.vector.tensor_mul(rot, rot, sin_s[:, t0:t0 + cols])
            nc.vector.tensor_add(us, tmp, rot)

        def proj(acc_tag, w_ap, xts, dst, c0):
            acc = ps1.tile([128, 512], f32, tag="pps", name=acc_tag)
            for d in range(ND):
                nc.tensor.matmul(acc, w_ap(d), xts[d],
                                 start=(d == 0), stop=(d == ND - 1))
            nc.scalar.copy(dst[:, c0:c0 + 512], acc)

        def scores_burst(h, qc):
            # kb high->low: diagonal (masked) tiles first so their gpsimd
            # affine_select latency hides behind the remaining exps
            qTh = qT_s[:, h * T + qc * 512:h * T + (qc + 1) * 512]
            nkb = 4 * qc + 4
            ptiles = {}
            for kb in reversed(range(nkb)):
                s_ps = sps.tile([128, 512], f32, tag="s",
                                name=f"s{h}_{qc}_{kb}")
                nc.tensor.matmul(
                    s_ps, kT_s[:, kb * 128:(kb + 1) * 128], qTh,
                    start=True, stop=True)
                p_sb = ppool.tile([128, 512], bf16, tag="p",
                                  name=f"p{h}_{qc}_{kb}")
                nc.scalar.activation(p_sb, s_ps, Exp, scale=SCALE)
                if kb >= 4 * qc:
                    nc.gpsimd.affine_select(
                        out=p_sb, in_=p_sb,
                        compare_op=mybir.AluOpType.is_ge,
                        fill=0.0, base=qc * 512 - kb * 128,
                        channel_multiplier=-1, pattern=[[1, 512]])
                ptiles[kb] = p_sb
            return ptiles

        def av_burst(h, qc, ptiles):
            nkb = 4 * qc + 4
            order = list(reversed(range(nkb)))
            oT = otp.tile([128, 512], f32, tag="oT", name=f"oT{h}_{qc}")
            dn = dnp.tile([128, 512], f32, tag="dn", name=f"dn{h}_{qc}")
            for i, kb in enumerate(order):
                nc.tensor.matmul(
                    oT, v_s[:, kb * HD:(kb + 1) * HD], ptiles[kb],
                    start=(i == 0), stop=(i == nkb - 1))
            for i, kb in enumerate(order):
                nc.tensor.matmul(
                    dn, ones_s, ptiles[kb],
                    start=(i == 0), stop=(i == nkb - 1))
            rec = rpool.tile([128, 512], f32, tag="rec")
            nc.vector.reciprocal_approx_fast(rec, dn)
            nc.vector.tensor_mul(
                aT_s[h][:, qc * 512:(qc + 1) * 512], oT, rec)

        def wo_block(qc, last=False):
            for tb in range(qc * 4, qc * 4 + 4):
                for n4 in range(4):
                    op = wops.tile([128, 512], f32, tag="op")
                    for h in range(HPC):
                        nc.tensor.matmul(
                            op, aT_s[h][:, tb * 128:(tb + 1) * 128],
                            wo_s[:, h, n4 * 512:(n4 + 1) * 512],
                            start=(h == 0), stop=(h == HPC - 1))
                    ob = ostage.tile([128, 512], bf16, tag="ob")
                    # last block alternates engines 1:1 so the copy chain
                    # pipelines across both (it's the kernel tail); earlier
                    # blocks stay DVE-heavy to keep ACT free for exps
                    on_act = ((tb * 4 + n4) % 2 == 1 if last
                              else (tb * 4 + n4) % 4 == 3)
                    if on_act:
                        nc.scalar.copy(ob, op)
                    else:
                        nc.vector.tensor_copy(ob, op)
                    nc.sync.dma_start(
                        out[tb * 128:(tb + 1) * 128,
                            n4 * 512:(n4 + 1) * 512], ob)

        # All x quarters preloaded up front (32 bufs, no slot recycling):
        # transfers drain the DMA engines by ~30us, before the v-transpose
        # DMAs (whose xbar-mode switch serializes the shared engines).
        for tq in range(1, 4):
            xq.append(load_x_quarter(tq))

        # Software pipeline with one-quarter lag between phases: proj
        # copies of quarter s queue ahead of quarter s-1's exps on ACT,
        # wo of s-2 rides along; x is fully preloaded so the v-transpose
        # DMAs (xbar-mode serialization) never stall the loads.
        for s in range(6):
            if s < 4:
                xts = xq[s]
                t0 = s * 512
                proj("k", lambda d: wk_s[:, d, :], xts, kT_s, t0)
                rope(kT_s, t0, t0)
                for h in range(HPC):
                    proj(f"q{h}",
                         lambda d, h=h: wq_s[:, d, h * HD:(h + 1) * HD],
                         xts, qT_s, h * T + t0)
                    rope(qT_s, h * T + t0, t0)
                proj("v", lambda d: wv_s[:, d, :], xts, vT_s, t0)
                # PE transposes (not DMA transpose: its xbar-mode switch
                # serializes the whole DMA subsystem ~1.2us each, stalling
                # the x loads and dropping the HAM clock)
                for tb in range(s * 4, s * 4 + 4):
                    vt = vtp.tile([128, 128], bf16, tag="vt")
                    nc.tensor.transpose(
                        vt, vT_s[:, tb * 128:(tb + 1) * 128], ident)
                    nc.scalar.copy(v_s[:, tb * HD:(tb + 1) * HD], vt)
            if 1 <= s <= 4:
                qc = s - 1
                pending = scores_burst(0, qc)
                for h in range(HPC):
                    nxt = scores_burst(1, qc) if h == 0 else None
                    av_burst(h, qc, pending)
                    pending = nxt
            if s >= 2:
                wo_block(s - 2, last=(s == 5))

    nc.compile()
    return nc


def _shard_inputs(x, wq, wk, wv, wo, cos, sin):
    import ml_dtypes
    bf = ml_dtypes.bfloat16

    def pdn(wT, n):
        # [DIM, n] row-major (d p) -> (p d): straight per-partition DMA bursts
        return np.ascontiguousarray(
            wT.reshape(ND, 128, n).transpose(1, 0, 2).reshape(DIM, n)
        ).astype(bf)

    xTh = np.ascontiguousarray(x.reshape(T, DIM).T).astype(bf)
    cosTh = np.ascontiguousarray(cos.T).astype(bf)
    # rotate_half sign fold: out = u*cos + u_rot*sin_signed
    sinTh = np.ascontiguousarray(sin.T).copy()
    sinTh[: HD // 2, :] *= -1.0
    sinTh = sinTh.astype(bf)
    in_maps = []
    for c in range(NCORES):
        g = c // 2
        woTc = wo[:, c * HPC * HD:(c + 1) * HPC * HD].T  # [(h p), DIM]
        in_maps.append({
            "xT": xTh,
            "wqT": pdn(wq[c * HPC * HD:(c + 1) * HPC * HD, :].T, HPC * HD),
            "wkT": pdn(wk[g * HD:(g + 1) * HD, :].T, HD),
            "wvT": pdn(wv[g * HD:(g + 1) * HD, :].T, HD),
            "woT": np.ascontiguousarray(
                woTc.reshape(HPC, 128, DIM).transpose(1, 0, 2).reshape(
                    HPC * HD, DIM)).astype(bf),
            "cosT": cosTh,
            "sinT": sinTh,
        })
    return in_maps


def _get_exec():
    """Build (once) a cached jitted SPMD executable over the 8 cores.

    Mirrors bass2jax.run_bass_via_pjrt's multi-core branch, but caches the
    jitted callable so repeat kernel() calls don't re-trace/re-lower.
    """
    if "exec" in _CACHE:
        return _CACHE["exec"]

    import jax
    from jax.sharding import Mesh, PartitionSpec
    from jax.experimental.shard_map import shard_map
    from concourse import bass2jax
    import concourse.mybir as mybir

    if "nc" not in _CACHE:
        _CACHE["nc"] = _build_nc()
    nc = _CACHE["nc"]

    bass2jax.install_neuronx_cc_hook()

    part_name = (nc.partition_id_tensor.name
                 if nc.partition_id_tensor else None)
    in_names, out_names, out_avals = [], [], []
    for alloc in nc.m.functions[0].allocations:
        if not isinstance(alloc, mybir.MemoryLocationSet):
            continue
        name = alloc.memorylocations[0].name
        if alloc.kind == "ExternalInput":
            if name != part_name:
                in_names.append(name)
        elif alloc.kind == "ExternalOutput":
            out_names.append(name)
            out_avals.append(jax.core.ShapedArray(
                tuple(alloc.tensor_shape), mybir.dt.np(alloc.dtype)))

    bind_names = in_names + out_names
    if part_name is not None:
        bind_names = bind_names + [part_name]

    def _body(*args):
        operands = list(args)
        if part_name is not None:
            operands.append(bass2jax.partition_id_tensor())
        outs = bass2jax._bass_exec_p.bind(
            *operands,
            out_avals=tuple(out_avals),
            in_names=tuple(bind_names),
            out_names=tuple(out_names),
            lowering_input_output_aliases=(),
            sim_require_finite=True,
            sim_require_nnan=True,
            nc=nc,
        )
        return tuple(outs)

    devices = jax.devices()[:NCORES]
    mesh = Mesh(np.asarray(devices), ("core",))
    n_in = len(in_names)
    n_out = len(out_names)
    sharded = jax.jit(
        shard_map(
            _body, mesh=mesh,
            in_specs=(PartitionSpec("core"),) * (n_in + n_out),
            out_specs=(PartitionSpec("core"),) * n_out,
            check_rep=False,
        ),
        donate_argnums=tuple(range(n_in, n_in + n_out)),
        keep_unused=True,
    )
    _CACHE["body"] = _body
    _CACHE["exec"] = (sharded, in_names, out_names, out_avals, mesh)
    return _CACHE["exec"]


def _concat_inputs(in_maps, in_names):
    return [
        np.concatenate([in_maps[c][name] for c in range(NCORES)], axis=0)
        for name in in_names
    ]


def _zero_outs(out_avals):
    return [
        np.zeros((NCORES * a.shape[0], *a.shape[1:]), a.dtype)
        for a in out_avals
    ]


def kernel(**inputs):
    sharded, in_names, out_names, out_avals, _ = _get_exec()

    in_maps = _shard_inputs(
        np.asarray(inputs["x"], dtype=np.float32),
        np.asarray(inputs["wq"], dtype=np.float32),
        np.asarray(inputs["wk"], dtype=np.float32),
        np.asarray(inputs["wv"], dtype=np.float32),
        np.asarray(inputs["wo"], dtype=np.float32),
        np.asarray(inputs["cos"], dtype=np.float32),
        np.asarray(inputs["sin"], dtype=np.float32),
    )
    concat_in = _concat_inputs(in_maps, in_names)
    out_arrs = sharded(*concat_in, *_zero_outs(out_avals))

    full = np.asarray(out_arrs[out_names.index("out")])
    acc = full.reshape(NCORES, T, DIM).astype(np.float32).sum(axis=0)
    return acc.reshape(1, T, DIM)
